# revision 8
# baseline (speedup 1.0000x reference)
"""Trainium2 Bass kernel: Tacotron-style location-sensitive attention step.

Sharding (8 NeuronCores, SPMD):
  - Attention / conv / softmax / context: batch parallel (16 ex/core).
  - LSTM: H-sharded; partial qry2 = h_shard @ Wq_shard.T for the full
    batch, then a bf16 ReduceScatter (issued from the sync queue so the
    gpsimd DMA stream never stalls) leaves each core with the final
    qry2 rows for its own 16 examples.

Key layout/perf choices:
  - All big streams (W, prenet/prev_ctx/att_h/att_c, conv windows,
    procT, enc) are f32->bf16 cast DMAs on the gpsimd SWDGE queue,
    ordered W-first, enc-last; W/input transposes run in bf16 through
    bitcast PSUM views (1 cycle/row).
  - Location features computed transposed: loc.T[a, s] via WfoldT
    (W_loc folded into conv weights), conv input as 62 overlapping
    window rows DMA'd from a host-padded [BL, 2, S+30] tensor.
  - proc_mem host-transposed to [BL, A, S]; qry2 becomes a per-
    partition ACT bias fused into the tanh.
  - scores: two N=512 matmuls per example into one shared PSUM bank at
    partitions {0,32,64,96} via tile_position; softmax is partition-
    parallel bf16; context = weights @ enc with bf16 enc tiles.

kernel(**inputs) takes FULL numpy inputs and returns [128, 512] f32.
"""

import sys

sys.path.insert(0, "/opt/trn_rl_repo")

import numpy as np

import concourse.bass as bass
import concourse.mybir as mybir
from concourse import bacc
from concourse.bass_utils import run_bass_kernel_spmd
from concourse.masks import make_identity
from concourse.tile import TileContext

F32 = mybir.dt.float32
BF16 = mybir.dt.bfloat16
AF = mybir.ActivationFunctionType

B, S, E, P, H, A, F, KW = 128, 1024, 512, 256, 1024, 128, 32, 31
NCORES = 8
BL = B // NCORES        # 16 examples per core
HL = H // NCORES        # 128 h rows per core
PE_DIM = P + E          # 768
NKI = PE_DIM // 128     # 6
NKH = H // 128          # 8
NC_S = S // 128         # 8 s-chunks
PADW = KW // 2          # 15
PADL = S + 2 * PADW     # 1054 padded conv row
TAPS = 2 * KW           # 62
ENC_BUFS = 13
XH = BL // 2            # 8 examples per xpad half


def build():
    nc = bacc.Bacc("TRN2", target_bir_lowering=False, debug=False,
                   num_devices=NCORES)

    dp = nc.declare_dram_parameter
    prenet = dp("prenet", [B, P], F32, isOutput=False)
    prev_ctx = dp("prev_ctx", [B, E], F32, isOutput=False)
    att_h = dp("att_h", [B, H], F32, isOutput=False)
    att_c_sh = dp("att_c_sh", [B, HL], F32, isOutput=False)
    w_ih_sh = dp("w_ih_sh", [4, HL, PE_DIM], F32, isOutput=False)
    w_hh_sh = dp("w_hh_sh", [4, HL, H], F32, isOutput=False)
    b_sh = dp("b_sh", [4, HL], F32, isOutput=False)
    loc_pad = dp("loc_pad", [BL, 2, PADL], F32, isOutput=False)
    enc = dp("enc", [BL, S, E], F32, isOutput=False)
    procT = dp("procT", [BL, A, S], F32, isOutput=False)
    conv_w = dp("conv_w", [F, 2, KW], F32, isOutput=False)
    cb_col = dp("cb_col", [F, 1], F32, isOutput=False)
    w_loc = dp("w_loc", [A, F], F32, isOutput=False)
    blq_col = dp("blq_col", [A, 1], F32, isOutput=False)
    wq_shT = dp("wq_shT", [HL, A], F32, isOutput=False)
    wo_colT = dp("wo_colT", [A, 1], F32, isOutput=False)
    out = dp("out", [BL, E], F32, isOutput=True)

    with TileContext(nc) as tc:
        with (
            tc.tile_pool(name="const", bufs=1) as cpool,
            tc.tile_pool(name="work", bufs=2) as wpool,
            tc.tile_pool(name="xpadp", bufs=1) as xpool,
            tc.tile_pool(name="proc", bufs=16) as ppool,
            tc.tile_pool(name="enc", bufs=ENC_BUFS) as epool,
            tc.tile_pool(name="psA", bufs=2, space="PSUM") as psA,
            tc.tile_pool(name="psL", bufs=1, space="PSUM") as psL,
            tc.tile_pool(name="psS", bufs=2, space="PSUM") as psS,
            tc.tile_pool(name="psX", bufs=2, space="PSUM") as psX,
            tc.tile_pool(name="dram", bufs=1, space="DRAM") as dpool,
        ):
            def mm_ps(shape):
                t = psA.tile([128, 512], F32, tag="mm")
                return t[: shape[0], : shape[1]]

            ident = cpool.tile([128, 128], F32)
            make_identity(nc, ident[:])
            ident_bf = cpool.tile([128, 128], BF16)
            nc.vector.tensor_copy(ident_bf[:], ident[:])

            pe_t_ctr = [0]

            def pick_engine(engine):
                if engine is None:
                    pe_t_ctr[0] += 1
                    engine = "dve" if pe_t_ctr[0] % 2 else "act"
                return engine

            def pe_t(dst, src_ap, rows, engine=None):
                """f32 transpose via TensorE (+copy/cast)."""
                ps = mm_ps((dst.shape[0], rows))
                nc.tensor.transpose(ps, src_ap, ident[:rows, :rows])
                if pick_engine(engine) == "dve":
                    nc.vector.tensor_copy(dst, ps)
                else:
                    nc.scalar.copy(dst, ps)

            def pe_t_multi_bf(dst_ap, srcs, rows, engine):
                """Up to 8 bf16 [*, rows<=128] transposes packed into one
                PSUM bank via a bitcast view, one wide copy out."""
                ps = psA.tile([128, 512], F32, tag="mm")
                pb = ps[:].bitcast(BF16)
                for i, s_ap in enumerate(srcs):
                    nc.tensor.transpose(pb[:, i * rows:(i + 1) * rows], s_ap,
                                        ident_bf[:rows, :rows])
                if pick_engine(engine) == "dve":
                    nc.vector.tensor_copy(dst_ap, pb[:, :len(srcs) * rows])
                else:
                    nc.scalar.copy(dst_ap, pb[:, :len(srcs) * rows])

            # ---- gpsimd (swdge) stream: W first, then acts, conv windows,
            #      procT, first enc tiles.
            NK = NKI + NKH  # 14
            wtpool_cm = tc.tile_pool(name="wt", bufs=1)
            wtpool = wtpool_cm.__enter__()
            wT = wtpool.tile([128, 4, NK, HL], BF16)
            wi_all = wtpool.tile([HL, 4, PE_DIM], BF16)
            nc.gpsimd.dma_start(wi_all[:],
                                w_ih_sh.rearrange("g hl k -> hl g k"))
            wh_all = wtpool.tile([HL, 4, H], BF16)
            nc.gpsimd.dma_start(wh_all[:],
                                w_hh_sh.rearrange("g hl k -> hl g k"))

            pn_nat = wtpool.tile([B, P], BF16)
            nc.gpsimd.dma_start(pn_nat[:], prenet[:])
            pc_nat = wtpool.tile([B, E], BF16)
            nc.gpsimd.dma_start(pc_nat[:], prev_ctx[:])
            ah_nat = wtpool.tile([B, H], BF16)
            nc.gpsimd.dma_start(ah_nat[:], att_h[:])
            ac_nat = wtpool.tile([B, HL], BF16)
            nc.gpsimd.dma_start(ac_nat[:], att_c_sh[:])

            def xpad_dma(hb):
                xp = xpool.tile([TAPS, XH, S], BF16, tag="xp")
                for c in range(2):
                    sl = loc_pad[XH * hb, c, 0:1]
                    src = bass.AP(
                        tensor=sl.tensor,
                        offset=sl.offset,
                        ap=[[1, KW], [2 * PADL, XH], [1, S]],
                    )
                    nc.gpsimd.dma_start(xp[c * KW:(c + 1) * KW], src)
                return xp

            xp1 = xpad_dma(0)

            proc_tiles = []
            for b in range(BL):
                pt = ppool.tile([A, S], BF16, tag="proc")
                nc.gpsimd.dma_start(pt[:], procT[b])
                proc_tiles.append(pt)

            enc_tiles = []
            for b in range(6):
                et = epool.tile([128, NC_S, E], BF16, tag="enc")
                nc.gpsimd.dma_start(
                    et[:], enc[b].rearrange("(p r) e -> p r e", r=NC_S))
                enc_tiles.append(et)

            # ---- small consts on the scalar (ACT hwdge) queue
            cw_nat = cpool.tile([F, TAPS], F32)
            nc.scalar.dma_start(cw_nat[:], conv_w.rearrange("f c k -> f (c k)"))
            wl_nat = cpool.tile([A, F], F32)
            nc.scalar.dma_start(wl_nat[:], w_loc[:])
            cb_sb = cpool.tile([F, 1], F32)
            nc.scalar.dma_start(cb_sb[:], cb_col[:])
            blq_sb = cpool.tile([A, 1], F32)
            nc.scalar.dma_start(blq_sb[:], blq_col[:])
            wq_nat = cpool.tile([HL, A], F32)
            nc.scalar.dma_start(wq_nat[:], wq_shT[:])
            wo_nat = cpool.tile([A, 1], F32)
            nc.scalar.dma_start(wo_nat[:], wo_colT[:])
            bs_nat = wtpool.tile([4, HL], F32)
            nc.sync.dma_start(bs_nat[:], b_sh[:])

            # ---- LSTM front (bf16 transposes, 8-packed)
            def tr_w(g):
                chunks = [wi_all[:, g, k * 128:(k + 1) * 128]
                          for k in range(NKI)]
                chunks += [wh_all[:, g, k * 128:(k + 1) * 128]
                           for k in range(NKH)]
                pe_t_multi_bf(wT[:, g, 0:8, :], chunks[0:8], HL, "dve")
                pe_t_multi_bf(wT[:, g, 8:14, :], chunks[8:14], HL, "act")

            for g in range(4):
                tr_w(g)

            inpT = wtpool.tile([128, NKI, B], BF16)
            ichunks = [pn_nat[:, k * 128:(k + 1) * 128] for k in range(2)]
            ichunks += [pc_nat[:, k * 128:(k + 1) * 128] for k in range(4)]
            pe_t_multi_bf(inpT[:, :, :], ichunks, B, "dve")
            ahT = wtpool.tile([128, NKH, B], BF16)
            achunks = [ah_nat[:, k * 128:(k + 1) * 128] for k in range(NKH)]
            pe_t_multi_bf(ahT[:, :, :], achunks, B, "act")
            acT = wtpool.tile([HL, B], BF16)
            pe_t_multi_bf(acT[:], [ac_nat[:]], B, "dve")
            bias_sb = wtpool.tile([HL, 4], F32)
            pe_t(bias_sb[:], bs_nat[:], 4, engine="act")

            gate_sb = []
            for g in range(4):
                ps = mm_ps((HL, B))
                for k in range(NKI):
                    nc.tensor.matmul(ps, wT[:, g, k, :], inpT[:, k, :],
                                     start=(k == 0), stop=False)
                for k in range(NKH):
                    nc.tensor.matmul(ps, wT[:, g, NKI + k, :], ahT[:, k, :],
                                     start=False, stop=(k == NKH - 1))
                sb = wtpool.tile([HL, B], BF16, tag=f"gate{g}")
                fn = AF.Tanh if g == 2 else AF.Sigmoid
                nc.scalar.activation(sb[:], ps, fn, bias=bias_sb[:, g:g + 1])
                gate_sb.append(sb)

            cT = wtpool.tile([HL, B], BF16)
            nc.vector.tensor_mul(cT[:], gate_sb[1][:], acT[:])
            tg = wtpool.tile([HL, B], BF16)
            nc.vector.tensor_mul(tg[:], gate_sb[0][:], gate_sb[2][:])
            nc.vector.tensor_add(cT[:], cT[:], tg[:])
            nc.scalar.activation(tg[:], cT[:], AF.Tanh)
            hT_sh = wtpool.tile([HL, B], BF16)
            nc.vector.tensor_mul(hT_sh[:], gate_sb[3][:], tg[:])

            # ---- partial qry2 (full batch) -> DRAM (bf16) -> ReduceScatter
            wq_bf = wtpool.tile([HL, A], BF16)
            nc.vector.tensor_copy(wq_bf[:], wq_nat[:])
            ps_q = mm_ps((B, A))
            nc.tensor.matmul(ps_q, hT_sh[:], wq_bf[:], start=True, stop=True)
            qp_sb = wtpool.tile([B, A], BF16)
            nc.vector.tensor_copy(qp_sb[:], ps_q)
            qp_dram = dpool.tile([B, A], BF16)
            nc.sync.dma_start(qp_dram[:], qp_sb[:])
            wtpool_cm.__exit__(None, None, None)

            qrs_dram = dpool.tile([BL, A], BF16)
            nc.gpsimd.collective_compute(
                "ReduceScatter",
                mybir.AluOpType.add,
                replica_groups=[list(range(NCORES))],
                ins=[qp_dram[:].opt()],
                outs=[qrs_dram[:].opt()],
            )

            # ---- gpsimd stream, block B (resumes once the RS fires)
            for b in range(6, 12):
                et = epool.tile([128, NC_S, E], BF16, tag="enc")
                nc.gpsimd.dma_start(
                    et[:], enc[b].rearrange("(p r) e -> p r e", r=NC_S))
                enc_tiles.append(et)
            xp2 = xpad_dma(1)
            for b in range(12, BL):
                et = epool.tile([128, NC_S, E], BF16, tag="enc")
                nc.gpsimd.dma_start(
                    et[:], enc[b].rearrange("(p r) e -> p r e", r=NC_S))
                enc_tiles.append(et)

            # ---- conv/location precompute
            wlocT = cpool.tile([F, A], F32)
            pe_t(wlocT[:], wl_nat[:], A, engine="dve")
            ps_f = mm_ps((TAPS, A))
            nc.tensor.matmul(ps_f, cw_nat[:], wlocT[:], start=True, stop=True)
            wfold = cpool.tile([TAPS, A], BF16)
            nc.scalar.copy(wfold[:], ps_f)
            ps_c = mm_ps((A, 1))
            nc.tensor.matmul(ps_c, wlocT[:], cb_sb[:], start=True, stop=True)
            constT = cpool.tile([A, 1], F32)
            nc.vector.tensor_add(constT[:], ps_c, blq_sb[:])
            wo_bf = cpool.tile([A, 1], BF16)
            nc.vector.tensor_copy(wo_bf[:], wo_nat[:])

            # ---- v_pre[b] = loc.T + proc.T  (in [A, S] layout, bf16)
            for b in range(BL):
                xp = xp1 if b < XH else xp2
                bb = b % XH
                ps = psL.tile([128, S], F32, tag="loc")
                nc.tensor.matmul(ps[:, 0:512], wfold[:], xp[:, bb, 0:512],
                                 start=True, stop=True)
                nc.tensor.matmul(ps[:, 512:1024], wfold[:], xp[:, bb, 512:1024],
                                 start=True, stop=True)
                nc.vector.tensor_add(proc_tiles[b][:], ps[:], proc_tiles[b][:])

            # ---- final qry2 rows for this core
            qrs_sb = cpool.tile([BL, A], BF16)
            nc.scalar.dma_start(qrs_sb[:], qrs_dram[:])
            ps_t = psA.tile([128, 512], F32, tag="mm")
            nc.tensor.transpose(ps_t[:].bitcast(BF16)[:, 0:BL], qrs_sb[:],
                                ident_bf[:BL, :BL])
            qry2T = cpool.tile([A, BL], F32)
            nc.scalar.copy(qry2T[:], ps_t[:].bitcast(BF16)[:, 0:BL])
            nc.vector.tensor_scalar_add(qry2T[:], qry2T[:], constT[:])

            # ---- tail: tanh -> scores -> group softmax -> context
            for g in range(BL // 4):
                bs = list(range(g * 4, (g + 1) * 4))
                ps_a = psS.tile([128, 512], F32, tag="sc")
                ps_b = psS.tile([128, 512], F32, tag="sc")
                for i, b in enumerate(bs):
                    pt = proc_tiles[b]
                    nc.scalar.activation(pt[:], pt[:], AF.Tanh,
                                         bias=qry2T[:, b:b + 1])
                    row = 32 * i
                    nc.tensor.matmul(ps_a[row:row + 1, :], wo_bf[:],
                                     pt[:, 0:512], start=True, stop=True,
                                     tile_position=(0, row))
                    nc.tensor.matmul(ps_b[row:row + 1, :], wo_bf[:],
                                     pt[:, 512:1024], start=True, stop=True,
                                     tile_position=(0, row))

                # softmax over s; rows {0,32,64,96} hold the 4 examples,
                # other partitions carry garbage that is never read.
                sc_g = wpool.tile([128, S], BF16, tag="scg")
                nc.vector.tensor_copy(sc_g[:, 0:512], ps_a[:])
                nc.scalar.copy(sc_g[:, 512:1024], ps_b[:])
                mx = wpool.tile([128, 1], F32, tag="mxg")
                nc.vector.reduce_max(mx[:], sc_g[:], axis=mybir.AxisListType.X)
                nc.vector.tensor_scalar_mul(mx[:], mx[:], -1.0)
                sums = wpool.tile([128, 1], F32, tag="smg")
                nc.scalar.activation(sc_g[:], sc_g[:], AF.Exp, bias=mx[:],
                                     accum_out=sums[:])
                rs = wpool.tile([128, 1], F32, tag="rsg")
                nc.vector.reciprocal(rs[:], sums[:])
                nc.vector.tensor_scalar_mul(sc_g[:], sc_g[:], rs[:])

                # weights back to [s-part, example]: wTt[:, c, i] at s=8p+c
                wTt = wpool.tile([128, NC_S, 4], BF16, tag="wtt")
                ps_w = psA.tile([128, 512], F32, tag="mm")
                pb_w = ps_w[:].bitcast(BF16)
                for c in range(NC_S):
                    nc.tensor.transpose(pb_w[:, c * 128:(c + 1) * 128],
                                        sc_g[:, c:S:NC_S], ident_bf[:, :])
                pr = pb_w.rearrange("p (c x) -> p c x", c=NC_S)
                nc.vector.tensor_copy(wTt[:], pr[:, :, 0:128:32])

                for i, b in enumerate(bs):
                    ps_x = psX.tile([1, E], F32, tag="ctx")
                    for c in range(NC_S):
                        nc.tensor.matmul(ps_x, wTt[:, c, i:i + 1],
                                         enc_tiles[b][:, c, :],
                                         start=(c == 0), stop=(c == NC_S - 1))
                    ctx_row = wpool.tile([1, E], F32, tag="ctxrow")
                    if i % 2 == 0:
                        nc.vector.tensor_copy(ctx_row[:], ps_x)
                    else:
                        nc.scalar.copy(ctx_row[:], ps_x)
                    nc.sync.dma_start(out[b:b + 1, :], ctx_row[:])

    nc.compile()
    return nc


_NC_CACHE = None


def _get_nc():
    global _NC_CACHE
    if _NC_CACHE is None:
        _NC_CACHE = build()
    return _NC_CACHE


def shard_inputs(prenet, prev_context, att_h, att_c, prev_weights, cum_weights,
                 enc_seq, proc_mem, mask, W_ih, W_hh, b_ih, b_hh, conv_w,
                 conv_b, W_loc, b_loc, W_q, b_q, W_out, **_unused):
    f = np.ascontiguousarray
    w_ih4 = np.asarray(W_ih, np.float32).reshape(4, H, PE_DIM)
    w_hh4 = np.asarray(W_hh, np.float32).reshape(4, H, H)
    b4 = (np.asarray(b_ih, np.float32)
          + np.asarray(b_hh, np.float32)).reshape(4, H)
    blq = (np.asarray(b_loc, np.float32).reshape(A)
           + np.asarray(b_q, np.float32).reshape(A)).reshape(A, 1)
    in_maps = []
    for j in range(NCORES):
        bj = slice(BL * j, BL * (j + 1))
        hj = slice(HL * j, HL * (j + 1))
        lp = np.zeros((BL, 2, PADL), np.float32)
        lp[:, 0, PADW:PADW + S] = np.asarray(cum_weights, np.float32)[bj]
        lp[:, 1, PADW:PADW + S] = np.asarray(prev_weights, np.float32)[bj]
        in_maps.append({
            "prenet": f(np.asarray(prenet, np.float32)),
            "prev_ctx": f(np.asarray(prev_context, np.float32)),
            "att_h": f(np.asarray(att_h, np.float32)),
            "att_c_sh": f(np.asarray(att_c, np.float32)[:, hj]),
            "w_ih_sh": f(w_ih4[:, hj]),
            "w_hh_sh": f(w_hh4[:, hj]),
            "b_sh": f(b4[:, hj]),
            "loc_pad": lp,
            "enc": f(np.asarray(enc_seq, np.float32)[bj]),
            "procT": f(np.asarray(proc_mem, np.float32)[bj].transpose(0, 2, 1)),
            "conv_w": f(np.asarray(conv_w, np.float32)),
            "cb_col": f(np.asarray(conv_b, np.float32).reshape(F, 1)),
            "w_loc": f(np.asarray(W_loc, np.float32)),
            "blq_col": blq,
            "wq_shT": f(np.asarray(W_q, np.float32)[:, hj].T),
            "wo_colT": f(np.asarray(W_out, np.float32).reshape(1, A).T),
        })
    return in_maps


def kernel(**inputs):
    assert not np.any(np.asarray(inputs["mask"])), \
        "kernel assumes mask == 0 (softmax-shift support not implemented)"
    nc = _get_nc()
    in_maps = shard_inputs(**inputs)
    res = run_bass_kernel_spmd(nc, in_maps, core_ids=list(range(NCORES)))
    return np.concatenate([res.results[j]["out"] for j in range(NCORES)],
                          axis=0)


if __name__ == "__main__":
    print("building...")
    _get_nc()
    print("built ok")


# revision 10
# speedup vs baseline: 1.1181x; 1.1181x over previous
"""Trainium2 Bass kernel: Tacotron-style location-sensitive attention step.

Sharding (8 NeuronCores, SPMD):
  - Attention / conv / softmax / context: batch parallel (16 ex/core).
  - LSTM: H-sharded; h.T shards AllGathered (32KB bf16), then each core
    computes qry2 for the full batch and selects its own 16 rows with a
    one-hot matmul (bsel) so the SPMD program stays core-uniform.

Key layout/perf choices:
  - All big streams are f32->bf16 cast DMAs on the gpsimd SWDGE queue.
    The ring allows ~8 outstanding DMAs, so the AllGather sits after
    enough large transfers that descriptor generation reaches it just
    as h lands; the stream never starves.
  - Conv windows (im2col) are materialized host-side as a [62, BL, S]
    layout tensor so the load is 2 DMAs x 62 descriptors of 32KB.
  - Location features computed transposed: loc.T[a, s] via WfoldT
    (W_loc folded into conv weights); proc_mem host-transposed to
    [BL, A, S]; qry2 becomes a per-partition ACT bias in the tanh.
  - scores: two N=512 matmuls per example into one shared PSUM bank at
    partitions {0,32,64,96} via tile_position; softmax partition-
    parallel bf16; context = weights @ enc with resident bf16 enc.

kernel(**inputs) takes FULL numpy inputs and returns [128, 512] f32.
"""

import sys

sys.path.insert(0, "/opt/trn_rl_repo")

import numpy as np

import concourse.bass as bass
import concourse.mybir as mybir
from concourse import bacc
from concourse.bass_utils import run_bass_kernel_spmd
from concourse.masks import make_identity
from concourse.tile import TileContext

F32 = mybir.dt.float32
BF16 = mybir.dt.bfloat16
AF = mybir.ActivationFunctionType

B, S, E, P, H, A, F, KW = 128, 1024, 512, 256, 1024, 128, 32, 31
NCORES = 8
BL = B // NCORES        # 16 examples per core
HL = H // NCORES        # 128 h rows per core
PE_DIM = P + E          # 768
NKI = PE_DIM // 128     # 6
NKH = H // 128          # 8
NC_S = S // 128         # 8 s-chunks
PADW = KW // 2          # 15
TAPS = 2 * KW           # 62
ENC_BUFS = 11


def build():
    nc = bacc.Bacc("TRN2", target_bir_lowering=False, debug=False,
                   num_devices=NCORES)

    dp = nc.declare_dram_parameter
    prenet = dp("prenet", [B, P], F32, isOutput=False)
    prev_ctx = dp("prev_ctx", [B, E], F32, isOutput=False)
    att_h = dp("att_h", [B, H], F32, isOutput=False)
    att_c_sh = dp("att_c_sh", [B, HL], F32, isOutput=False)
    w_ih_sh = dp("w_ih_sh", [4, HL, PE_DIM], F32, isOutput=False)
    w_hh_sh = dp("w_hh_sh", [4, HL, H], F32, isOutput=False)
    b_sh = dp("b_sh", [4, HL], F32, isOutput=False)
    loc_win = dp("loc_win", [TAPS, BL, S], F32, isOutput=False)
    enc = dp("enc", [BL, S, E], F32, isOutput=False)
    procT = dp("procT", [BL, A, S], F32, isOutput=False)
    conv_w = dp("conv_w", [F, 2, KW], F32, isOutput=False)
    cb_col = dp("cb_col", [F, 1], F32, isOutput=False)
    w_loc = dp("w_loc", [A, F], F32, isOutput=False)
    blq_col = dp("blq_col", [A, 1], F32, isOutput=False)
    wqT_full = dp("wqT_full", [H, A], F32, isOutput=False)
    wo_colT = dp("wo_colT", [A, 1], F32, isOutput=False)
    bsel = dp("bsel", [B, BL], F32, isOutput=False)
    out = dp("out", [BL, E], F32, isOutput=True)

    with TileContext(nc) as tc:
        with (
            tc.tile_pool(name="const", bufs=1) as cpool,
            tc.tile_pool(name="work", bufs=2) as wpool,
            tc.tile_pool(name="xpadp", bufs=1) as xpool,
            tc.tile_pool(name="proc", bufs=16) as ppool,
            tc.tile_pool(name="enc", bufs=ENC_BUFS) as epool,
            tc.tile_pool(name="psA", bufs=2, space="PSUM") as psA,
            tc.tile_pool(name="psL", bufs=1, space="PSUM") as psL,
            tc.tile_pool(name="psS", bufs=2, space="PSUM") as psS,
            tc.tile_pool(name="psX", bufs=2, space="PSUM") as psX,
            tc.tile_pool(name="dram", bufs=1, space="DRAM") as dpool,
        ):
            def mm_ps(shape):
                t = psA.tile([128, 512], F32, tag="mm")
                return t[: shape[0], : shape[1]]

            ident = cpool.tile([128, 128], F32)
            make_identity(nc, ident[:])
            ident_bf = cpool.tile([128, 128], BF16)
            nc.vector.tensor_copy(ident_bf[:], ident[:])

            pe_t_ctr = [0]

            def pick_engine(engine):
                if engine is None:
                    pe_t_ctr[0] += 1
                    engine = "dve" if pe_t_ctr[0] % 2 else "act"
                return engine

            def pe_t(dst, src_ap, rows, engine=None):
                """f32 transpose via TensorE (+copy/cast)."""
                ps = mm_ps((dst.shape[0], rows))
                nc.tensor.transpose(ps, src_ap, ident[:rows, :rows])
                if pick_engine(engine) == "dve":
                    nc.vector.tensor_copy(dst, ps)
                else:
                    nc.scalar.copy(dst, ps)

            def pe_t_multi_bf(dst_ap, srcs, rows, engine):
                """Up to 8 bf16 transposes packed into one PSUM bank via a
                bitcast view, one wide copy out."""
                ps = psA.tile([128, 512], F32, tag="mm")
                pb = ps[:].bitcast(BF16)
                for i, s_ap in enumerate(srcs):
                    nc.tensor.transpose(pb[:, i * rows:(i + 1) * rows], s_ap,
                                        ident_bf[:rows, :rows])
                if pick_engine(engine) == "dve":
                    nc.vector.tensor_copy(dst_ap, pb[:, :len(srcs) * rows])
                else:
                    nc.scalar.copy(dst_ap, pb[:, :len(srcs) * rows])

            # ---- gpsimd (swdge) stream, block A. Order matters: the ring
            # holds ~8 outstanding DMAs, and the AllGather below must be
            # reached by the generator right as h becomes ready.
            NK = NKI + NKH  # 14
            wtpool_cm = tc.tile_pool(name="wt", bufs=1)
            wtpool = wtpool_cm.__enter__()
            wT = wtpool.tile([128, 4, NK, HL], BF16)
            wi_all = wtpool.tile([HL, 4, PE_DIM], BF16)
            nc.gpsimd.dma_start(wi_all[:],
                                w_ih_sh.rearrange("g hl k -> hl g k"))
            wh_all = wtpool.tile([HL, 4, H], BF16)
            nc.gpsimd.dma_start(wh_all[:],
                                w_hh_sh.rearrange("g hl k -> hl g k"))

            pn_nat = wtpool.tile([B, P], BF16)
            nc.gpsimd.dma_start(pn_nat[:], prenet[:])
            pc_nat = wtpool.tile([B, E], BF16)
            nc.gpsimd.dma_start(pc_nat[:], prev_ctx[:])
            ah_nat = wtpool.tile([B, H], BF16)
            nc.gpsimd.dma_start(ah_nat[:], att_h[:])
            ac_nat = wtpool.tile([B, HL], BF16)
            nc.gpsimd.dma_start(ac_nat[:], att_c_sh[:])
            wqT_bf = cpool.tile([128, NKH, A], BF16)
            nc.gpsimd.dma_start(
                wqT_bf[:], wqT_full.rearrange("(c p) a -> p c a", p=128))

            xp = xpool.tile([TAPS, BL, S], BF16, tag="xp")
            nc.gpsimd.dma_start(xp[:, 0:8, :], loc_win[:, 0:8, :])
            nc.gpsimd.dma_start(xp[:, 8:BL, :], loc_win[:, 8:BL, :])

            proc_tiles = []
            for b in range(8):
                pt = ppool.tile([A, S], BF16, tag="proc")
                nc.gpsimd.dma_start(pt[:], procT[b])
                proc_tiles.append(pt)

            enc_tiles = []
            for b in range(6):
                et = epool.tile([128, NC_S, E], BF16, tag="enc")
                nc.gpsimd.dma_start(
                    et[:], enc[b].rearrange("(p r) e -> p r e", r=NC_S))
                enc_tiles.append(et)

            # ---- small consts on the scalar (ACT hwdge) queue
            cw_nat = cpool.tile([F, TAPS], F32)
            nc.scalar.dma_start(cw_nat[:], conv_w.rearrange("f c k -> f (c k)"))
            wl_nat = cpool.tile([A, F], F32)
            nc.scalar.dma_start(wl_nat[:], w_loc[:])
            cb_sb = cpool.tile([F, 1], F32)
            nc.scalar.dma_start(cb_sb[:], cb_col[:])
            blq_sb = cpool.tile([A, 1], F32)
            nc.scalar.dma_start(blq_sb[:], blq_col[:])
            wo_nat = cpool.tile([A, 1], F32)
            nc.scalar.dma_start(wo_nat[:], wo_colT[:])
            sel_nat = cpool.tile([B, BL], F32)
            nc.scalar.dma_start(sel_nat[:], bsel[:])
            sel_sb = cpool.tile([B, BL], BF16)
            nc.vector.tensor_copy(sel_sb[:], sel_nat[:])
            bs_nat = wtpool.tile([4, HL], F32)
            nc.sync.dma_start(bs_nat[:], b_sh[:])

            # ---- LSTM front (bf16 transposes, 8-packed)
            def tr_w(g):
                chunks = [wi_all[:, g, k * 128:(k + 1) * 128]
                          for k in range(NKI)]
                chunks += [wh_all[:, g, k * 128:(k + 1) * 128]
                           for k in range(NKH)]
                pe_t_multi_bf(wT[:, g, 0:8, :], chunks[0:8], HL, "dve")
                pe_t_multi_bf(wT[:, g, 8:14, :], chunks[8:14], HL, "act")

            for g in range(4):
                tr_w(g)

            inpT = wtpool.tile([128, NKI, B], BF16)
            ichunks = [pn_nat[:, k * 128:(k + 1) * 128] for k in range(2)]
            ichunks += [pc_nat[:, k * 128:(k + 1) * 128] for k in range(4)]
            pe_t_multi_bf(inpT[:, :, :], ichunks, B, "dve")
            ahT = wtpool.tile([128, NKH, B], BF16)
            achunks = [ah_nat[:, k * 128:(k + 1) * 128] for k in range(NKH)]
            pe_t_multi_bf(ahT[:, :, :], achunks, B, "act")
            acT = wtpool.tile([HL, B], BF16)
            pe_t_multi_bf(acT[:], [ac_nat[:]], B, "dve")
            bias_sb = wtpool.tile([HL, 4], F32)
            pe_t(bias_sb[:], bs_nat[:], 4, engine="act")

            gate_sb = []
            for g in range(4):
                ps = mm_ps((HL, B))
                for k in range(NKI):
                    nc.tensor.matmul(ps, wT[:, g, k, :], inpT[:, k, :],
                                     start=(k == 0), stop=False)
                for k in range(NKH):
                    nc.tensor.matmul(ps, wT[:, g, NKI + k, :], ahT[:, k, :],
                                     start=False, stop=(k == NKH - 1))
                sb = wtpool.tile([HL, B], BF16, tag=f"gate{g}")
                fn = AF.Tanh if g == 2 else AF.Sigmoid
                nc.scalar.activation(sb[:], ps, fn, bias=bias_sb[:, g:g + 1])
                gate_sb.append(sb)

            cT = wtpool.tile([HL, B], BF16)
            nc.vector.tensor_mul(cT[:], gate_sb[1][:], acT[:])
            tg = wtpool.tile([HL, B], BF16)
            nc.vector.tensor_mul(tg[:], gate_sb[0][:], gate_sb[2][:])
            nc.vector.tensor_add(cT[:], cT[:], tg[:])
            nc.scalar.activation(tg[:], cT[:], AF.Tanh)
            hT_sh = wtpool.tile([HL, B], BF16)
            nc.vector.tensor_mul(hT_sh[:], gate_sb[3][:], tg[:])
            h_in = dpool.tile([HL, B], BF16)
            nc.sync.dma_start(h_in[:], hT_sh[:])
            wtpool_cm.__exit__(None, None, None)

            # ---- AllGather h.T shards (fires as soon as h_in lands)
            h_gat = dpool.tile([NCORES, HL, B], BF16)
            nc.gpsimd.collective_compute(
                "AllGather",
                mybir.AluOpType.bypass,
                replica_groups=[list(range(NCORES))],
                ins=[h_in[:].opt()],
                outs=[h_gat[:].opt()],
            )

            # ---- gpsimd stream, block B (resumes once the AG fires)
            for b in range(8, BL):
                pt = ppool.tile([A, S], BF16, tag="proc")
                nc.gpsimd.dma_start(pt[:], procT[b])
                proc_tiles.append(pt)
            for b in range(6, BL):
                et = epool.tile([128, NC_S, E], BF16, tag="enc")
                nc.gpsimd.dma_start(
                    et[:], enc[b].rearrange("(p r) e -> p r e", r=NC_S))
                enc_tiles.append(et)

            # ---- conv/location precompute
            wlocT = cpool.tile([F, A], F32)
            pe_t(wlocT[:], wl_nat[:], A, engine="dve")
            ps_f = mm_ps((TAPS, A))
            nc.tensor.matmul(ps_f, cw_nat[:], wlocT[:], start=True, stop=True)
            wfold = cpool.tile([TAPS, A], BF16)
            nc.scalar.copy(wfold[:], ps_f)
            ps_c = mm_ps((A, 1))
            nc.tensor.matmul(ps_c, wlocT[:], cb_sb[:], start=True, stop=True)
            constT = cpool.tile([A, 1], F32)
            nc.vector.tensor_add(constT[:], ps_c, blq_sb[:])
            wo_bf = cpool.tile([A, 1], BF16)
            nc.vector.tensor_copy(wo_bf[:], wo_nat[:])

            # ---- v_pre[b] = loc.T + proc.T  (in [A, S] layout, bf16)
            for b in range(BL):
                ps = psL.tile([128, S], F32, tag="loc")
                nc.tensor.matmul(ps[:, 0:512], wfold[:], xp[:, b, 0:512],
                                 start=True, stop=True)
                nc.tensor.matmul(ps[:, 512:1024], wfold[:], xp[:, b, 512:1024],
                                 start=True, stop=True)
                nc.vector.tensor_add(proc_tiles[b][:], ps[:], proc_tiles[b][:])

            # ---- qry2 for the full batch, then one-hot select this core's
            # 16 rows (transposed): qry2T[a, i]
            hfull = cpool.tile([128, NKH, B], BF16)
            nc.scalar.dma_start(hfull[:], h_gat[:].rearrange("c p b -> p c b"))
            ps_q = mm_ps((B, A))
            for k in range(NKH):
                nc.tensor.matmul(ps_q, hfull[:, k, :], wqT_bf[:, k, :],
                                 start=(k == 0), stop=(k == NKH - 1))
            qry2_sb = cpool.tile([B, A], BF16)
            nc.vector.tensor_copy(qry2_sb[:], ps_q)
            ps_t = mm_ps((A, BL))
            nc.tensor.matmul(ps_t, qry2_sb[:], sel_sb[:], start=True, stop=True)
            qry2T = cpool.tile([A, BL], F32)
            nc.scalar.copy(qry2T[:], ps_t)
            nc.vector.tensor_scalar_add(qry2T[:], qry2T[:], constT[:])

            # ---- tail: tanh -> scores -> group softmax -> context
            for g in range(BL // 4):
                bs = list(range(g * 4, (g + 1) * 4))
                ps_a = psS.tile([128, 512], F32, tag="sc")
                ps_b = psS.tile([128, 512], F32, tag="sc")
                for i, b in enumerate(bs):
                    pt = proc_tiles[b]
                    nc.scalar.activation(pt[:], pt[:], AF.Tanh,
                                         bias=qry2T[:, b:b + 1])
                    row = 32 * i
                    nc.tensor.matmul(ps_a[row:row + 1, :], wo_bf[:],
                                     pt[:, 0:512], start=True, stop=True,
                                     tile_position=(0, row))
                    nc.tensor.matmul(ps_b[row:row + 1, :], wo_bf[:],
                                     pt[:, 512:1024], start=True, stop=True,
                                     tile_position=(0, row))

                # softmax over s; rows {0,32,64,96} hold the 4 examples,
                # other partitions carry garbage that is never read.
                sc_g = wpool.tile([128, S], BF16, tag="scg")
                nc.vector.tensor_copy(sc_g[:, 0:512], ps_a[:])
                nc.scalar.copy(sc_g[:, 512:1024], ps_b[:])
                mx = wpool.tile([128, 1], F32, tag="mxg")
                nc.vector.reduce_max(mx[:], sc_g[:], axis=mybir.AxisListType.X)
                nc.vector.tensor_scalar_mul(mx[:], mx[:], -1.0)
                sums = wpool.tile([128, 1], F32, tag="smg")
                nc.scalar.activation(sc_g[:], sc_g[:], AF.Exp, bias=mx[:],
                                     accum_out=sums[:])
                rs = wpool.tile([128, 1], F32, tag="rsg")
                nc.vector.reciprocal(rs[:], sums[:])
                nc.vector.tensor_scalar_mul(sc_g[:], sc_g[:], rs[:])

                # weights back to [s-part, example]: wTt[:, c, i] at s=8p+c
                wTt = wpool.tile([128, NC_S, 4], BF16, tag="wtt")
                ps_w = psA.tile([128, 512], F32, tag="mm")
                pb_w = ps_w[:].bitcast(BF16)
                for c in range(NC_S):
                    nc.tensor.transpose(pb_w[:, c * 128:(c + 1) * 128],
                                        sc_g[:, c:S:NC_S], ident_bf[:, :])
                pr = pb_w.rearrange("p (c x) -> p c x", c=NC_S)
                nc.vector.tensor_copy(wTt[:], pr[:, :, 0:128:32])

                for i, b in enumerate(bs):
                    ps_x = psX.tile([1, E], F32, tag="ctx")
                    for c in range(NC_S):
                        nc.tensor.matmul(ps_x, wTt[:, c, i:i + 1],
                                         enc_tiles[b][:, c, :],
                                         start=(c == 0), stop=(c == NC_S - 1))
                    ctx_row = wpool.tile([1, E], F32, tag="ctxrow")
                    if i % 2 == 0:
                        nc.vector.tensor_copy(ctx_row[:], ps_x)
                    else:
                        nc.scalar.copy(ctx_row[:], ps_x)
                    nc.sync.dma_start(out[b:b + 1, :], ctx_row[:])

    nc.compile()
    return nc


_NC_CACHE = None


def _get_nc():
    global _NC_CACHE
    if _NC_CACHE is None:
        _NC_CACHE = build()
    return _NC_CACHE


def shard_inputs(prenet, prev_context, att_h, att_c, prev_weights, cum_weights,
                 enc_seq, proc_mem, mask, W_ih, W_hh, b_ih, b_hh, conv_w,
                 conv_b, W_loc, b_loc, W_q, b_q, W_out, **_unused):
    f = np.ascontiguousarray
    w_ih4 = np.asarray(W_ih, np.float32).reshape(4, H, PE_DIM)
    w_hh4 = np.asarray(W_hh, np.float32).reshape(4, H, H)
    b4 = (np.asarray(b_ih, np.float32)
          + np.asarray(b_hh, np.float32)).reshape(4, H)
    blq = (np.asarray(b_loc, np.float32).reshape(A)
           + np.asarray(b_q, np.float32).reshape(A)).reshape(A, 1)
    cum = np.asarray(cum_weights, np.float32)
    prv = np.asarray(prev_weights, np.float32)
    in_maps = []
    for j in range(NCORES):
        bj = slice(BL * j, BL * (j + 1))
        hj = slice(HL * j, HL * (j + 1))
        # host-side im2col: loc_win[c*31+k, b, s] = x_c[b, s + k - 15]
        lp = np.zeros((BL, 2, S + 2 * PADW), np.float32)
        lp[:, 0, PADW:PADW + S] = cum[bj]
        lp[:, 1, PADW:PADW + S] = prv[bj]
        st = lp.strides
        win = np.lib.stride_tricks.as_strided(
            lp, shape=(2, KW, BL, S), strides=(st[1], st[2], st[0], st[2]))
        loc_win = f(win.reshape(TAPS, BL, S))
        sel = np.zeros((B, BL), np.float32)
        sel[BL * j:BL * (j + 1), :] = np.eye(BL, dtype=np.float32)
        in_maps.append({
            "prenet": f(np.asarray(prenet, np.float32)),
            "prev_ctx": f(np.asarray(prev_context, np.float32)),
            "att_h": f(np.asarray(att_h, np.float32)),
            "att_c_sh": f(np.asarray(att_c, np.float32)[:, hj]),
            "w_ih_sh": f(w_ih4[:, hj]),
            "w_hh_sh": f(w_hh4[:, hj]),
            "b_sh": f(b4[:, hj]),
            "loc_win": loc_win,
            "enc": f(np.asarray(enc_seq, np.float32)[bj]),
            "procT": f(np.asarray(proc_mem, np.float32)[bj].transpose(0, 2, 1)),
            "conv_w": f(np.asarray(conv_w, np.float32)),
            "cb_col": f(np.asarray(conv_b, np.float32).reshape(F, 1)),
            "w_loc": f(np.asarray(W_loc, np.float32)),
            "blq_col": blq,
            "wqT_full": f(np.asarray(W_q, np.float32).T),
            "wo_colT": f(np.asarray(W_out, np.float32).reshape(1, A).T),
            "bsel": sel,
        })
    return in_maps


def kernel(**inputs):
    assert not np.any(np.asarray(inputs["mask"])), \
        "kernel assumes mask == 0 (softmax-shift support not implemented)"
    nc = _get_nc()
    in_maps = shard_inputs(**inputs)
    res = run_bass_kernel_spmd(nc, in_maps, core_ids=list(range(NCORES)))
    return np.concatenate([res.results[j]["out"] for j in range(NCORES)],
                          axis=0)


if __name__ == "__main__":
    print("building...")
    _get_nc()
    print("built ok")
